# revision 17
# baseline (speedup 1.0000x reference)
"""Trainium2 kernel for nn_DeepPatchEncoder.

The reference pipeline (patchify16 + pos_emb -> unpatchify -> patchify8 +
pos_new -> unpatchify -> patchify16) collapses algebraically: patchify /
unpatchify are inverse permutations, so

    out = patchify16(X + Z),   Z = unpatchify16(pos_emb) + unpatchify8(pos_new)

where Z is a single [224,224,3] image computed from the tiny parameters
(pos_emb conv + batchnorm).  Z is computed on host in numpy (O(100KB) of
work); the per-sample memory-bound add + patch permutation runs on 8
NeuronCores, data-parallel over the batch (16 samples per core).

Per core the work is 224 independent blocks (sample b x coarse row i).
Block input = 16 consecutive image rows (10752 floats, contiguous in
DRAM); block output = 14 consecutive encoder rows (10752 floats,
contiguous in DRAM).  Within a block the map is a pure (p0:16 <-> j:14)
axis swap of 48-float chunks, done on the VectorEngine as tensor_tensor
adds with strided access patterns (which also add Z).

Engine / queue layout (all HWDGE DMAs share one SDMA queue row, so
side traffic must ride SWDGE to overlap the main read stream):
  - x loads: 4 contiguous 2.4MB DMAs (2 tiles x 2 p0-halves) on the SP
    HWDGE ring, issued first -- the only traffic on that queue row.
  - z replication (zrep[p] = z[p % 14] across 112 partitions): one-hot
    selection matmul on the otherwise-idle TensorEngine.  fp32 matmuls
    are ~5x slower than bf16, so the host splits z into two bf16
    components (z ~ z0 + z1, ~6e-7 relative on the output) and the PE
    accumulates the two exact bf16 products in PSUM; ScalarEngine
    copies PSUM->SBUF.  The host pre-permutes z into (j-half, p0-half)
    quarter-major order so each quarter is a contiguous column range:
    z component loads are contiguous, and each TT depends on exactly
    one quarter (so the DVE starts ~10us in, not after the full
    replication).  z loads ride SWDGE (gpsimd).
  - TTs: 8 per core ([112, 2688] each), consume (tile, j-half, p0-half).
  - stores: 4 contiguous 2.4MB DMAs (j-halves) on SWDGE, overlapping
    the read stream at the SDMA-engine round-robin level.
"""
import sys

for _p in ("/opt/trn_rl_repo", "/root/.axon_site/_ro/trn_rl_repo",
           "/root/.axon_site/_ro/pypackages"):
    if _p not in sys.path:
        sys.path.append(_p)

import numpy as np
import ml_dtypes
import concourse.bass as bass
import concourse.bacc as bacc
import concourse.mybir as mybir
import concourse.tile as tile
from concourse.bass_utils import run_bass_kernel_spmd

F32 = mybir.dt.float32
BF16 = mybir.dt.bfloat16

B, IMG, C = 128, 224, 3
P0, P1 = 16, 8
N0 = (IMG // P0) ** 2   # 196
D0 = C * P0 * P0        # 768
BN_EPS = 1e-3

NCORES = 8
NB = B // NCORES        # 16 samples per core
NI = IMG // P0          # 14 coarse rows
NBLK = NB * NI          # 224 blocks per core
ROWF = IMG * C          # 672 floats per image row
FREE = P0 * ROWF        # 10752 floats per block
P = 112                 # partitions per tile
NT = NBLK // P          # 2 tiles
NH = 2                  # j-halves (store / TT granularity)
JH = NI // NH           # 7
HFREE = FREE // NH      # 5376
NP0H = 2                # p0-halves (load / TT granularity)
P0H = P0 // NP0H        # 8
PHF = FREE // NP0H      # 5376 floats per p0-half (contiguous in x)
NQ = NH * NP0H          # 4 z quarters
QF = FREE // NQ         # 2688 floats per quarter
NZC = 2                 # bf16 z components
MMN = 512               # matmul moving-dim tile


def _compute_z(pos_emb, conv_w, bn_gamma, bn_beta, bn_mean, bn_var):
    """The [224,224,3] constant image Z (all-numpy, host side)."""
    pos_emb = np.asarray(pos_emb, np.float32)
    # unpatchify16(pos_emb): [196,768] -> [224,224,3]
    q = pos_emb.reshape(14, 14, P0, P0, C).transpose(0, 2, 1, 3, 4)
    q = q.reshape(IMG, IMG, C)

    # pos pipeline: [3,16,16,196] -conv2x2s2-> [3,8,8,784] -> BN
    pos_img = pos_emb.reshape(N0, P0, P0, C).transpose(3, 1, 2, 0)
    v = pos_img.reshape(C, 8, 2, 8, 2, N0).astype(np.float64)
    pos_c = np.einsum("nidjec,deco->nijo", v, np.asarray(conv_w, np.float64))
    inv = np.asarray(bn_gamma, np.float64) / np.sqrt(
        np.asarray(bn_var, np.float64) + BN_EPS)
    pos_c = (pos_c - np.asarray(bn_mean, np.float64)) * inv + np.asarray(
        bn_beta, np.float64)
    pos_new = pos_c.transpose(3, 1, 2, 0).astype(np.float32)  # [784,8,8,3]

    # unpatchify8(pos_new): [784,8,8,3] -> [224,224,3]
    r = pos_new.reshape(28, 28, P1, P1, C).transpose(0, 2, 1, 3, 4)
    r = r.reshape(IMG, IMG, C)
    return q + r


def _quarter_major(z):
    """[14, (p0:16, j:14, k:48)] -> [14, (h, ph, p0l:8, jl:7, k:48)].

    Quarter (h, ph) becomes the contiguous column range
    [(h*2+ph)*QF, (h*2+ph+1)*QF), laid out (p0l, jl, k)."""
    v = z.reshape(NI, NP0H, P0H, NH, JH, 48)        # i, ph, p0l, h, jl, k
    return np.ascontiguousarray(
        v.transpose(0, 3, 1, 2, 4, 5).reshape(NI, FREE))


_NC_CACHE = None


def _build_kernel():
    global _NC_CACHE
    if _NC_CACHE is not None:
        return _NC_CACHE
    nc = bacc.Bacc()
    x = nc.declare_dram_parameter("x", [NBLK, FREE], F32, isOutput=False)
    # z components, pre-permuted to quarter-major layout on host
    zs = [nc.declare_dram_parameter(f"z{i}", [NI, FREE], BF16, isOutput=False)
          for i in range(NZC)]
    s = nc.declare_dram_parameter("s", [NI, P], BF16, isOutput=False)
    out = nc.declare_dram_parameter("out", [NBLK, FREE], F32, isOutput=True)

    with tile.TileContext(nc) as tc:
        with (
            tc.tile_pool(name="cpool", bufs=1) as cpool,
            tc.tile_pool(name="zck", bufs=6) as zck,
            tc.tile_pool(name="zp", bufs=1) as zp,
            tc.tile_pool(name="ps", bufs=4, space="PSUM") as ps,
            tc.tile_pool(name="xp", bufs=2) as xp,
            tc.tile_pool(name="op", bufs=2) as op,
        ):
            # HWDGE (SP ring, queue row 1) carries the reads: the small
            # s/z-component loads sit at the head, interleaved with the
            # four fat x sub-loads so early z quarters arrive early.
            # (SWDGE's first DMA pays a ~13us GPSIMD library load, so the
            # latency-critical z loads must not ride it.)
            s_tile = cpool.tile([NI, P], BF16)
            nc.sync.dma_start(out=s_tile[:], in_=s[:, :])
            xts = [xp.tile([P, FREE], F32, tag="xt", name=f"xt{t}")
                   for t in range(NT)]
            zq_tiles = []
            zc_per_q = [[] for _ in range(NQ)]

            def load_zq(qi):
                for i in range(NZC):
                    zc = zck.tile([NI, QF], BF16, tag="zc")
                    nc.sync.dma_start(
                        out=zc[:], in_=zs[i][:, qi * QF:(qi + 1) * QF])
                    zc_per_q[qi].append(zc)

            def load_x(t, ph):
                nc.sync.dma_start(
                    out=xts[t][:, ph * PHF:(ph + 1) * PHF],
                    in_=x[t * P:(t + 1) * P, ph * PHF:(ph + 1) * PHF])

            # ring order: early z quarters first; later ones slotted
            # between the fat x sub-loads (zck has 6 slots so the q3
            # loads can't block the x loads queued behind them)
            load_zq(0)
            load_zq(1)
            load_x(0, 0)
            load_zq(2)
            load_x(0, 1)
            load_zq(3)
            load_x(1, 0)
            load_x(1, 1)

            # z replication (zrep[p] = z[p % 14]) on the TensorEngine:
            # psum[112, n] = S.T @ z_chunk (S one-hot bf16, exact),
            # accumulating the two bf16 z components.  Quarter at a time,
            # in TT consumption order.
            for qi in range(NQ):
                zqt = zp.tile([P, QF], F32, tag=f"zq{qi}")
                zq_tiles.append(zqt)
                zcs = zc_per_q[qi]
                for c0 in range(0, QF, MMN):
                    n = min(MMN, QF - c0)
                    pz = ps.tile([P, MMN], F32, tag="pz")
                    for i in range(NZC):
                        nc.tensor.matmul(pz[:, :n], s_tile[:],
                                         zcs[i][:, c0:c0 + n],
                                         start=(i == 0), stop=(i == NZC - 1))
                    nc.scalar.copy(out=zqt[:, c0:c0 + n], in_=pz[:, :n])

            for t in range(NT):
                xt = xts[t]
                for h in range(NH):
                    ot = op.tile([P, HFREE], F32, tag="ot")
                    for ph in range(NP0H):
                        # input view: (j:7, p0:8, k:48) strided over the
                        # p0-half of xt
                        in0 = xt[:].rearrange(
                            "p (p0 j k) -> p j p0 k", p0=P0, j=NI, k=48)[
                            :, h * JH:(h + 1) * JH,
                            ph * P0H:(ph + 1) * P0H]
                        # zrep quarter laid out (p0l:8, jl:7, k:48)
                        in1 = zq_tiles[h * NP0H + ph][:].rearrange(
                            "p (p0 j k) -> p j p0 k", p0=P0H, j=JH, k=48)
                        # output view inside the j-half tile
                        o0 = ot[:].rearrange(
                            "p (j p0 k) -> p j p0 k", j=JH, p0=P0, k=48)[
                            :, :, ph * P0H:(ph + 1) * P0H]
                        nc.vector.tensor_tensor(o0, in0, in1,
                                                mybir.AluOpType.add)
                    nc.gpsimd.dma_start(
                        out=out[t * P:(t + 1) * P, h * HFREE:(h + 1) * HFREE],
                        in_=ot[:])
    nc.finalize()
    _NC_CACHE = nc
    return nc


_S_NP = np.zeros((NI, P), ml_dtypes.bfloat16)
for _pp in range(P):
    _S_NP[_pp % NI, _pp] = 1.0


def _split_bf16(z, k=NZC):
    """z (f32) -> k bf16 arrays summing to z up to ~2^-(9k) relative."""
    parts = []
    r = z.astype(np.float32)
    for _ in range(k):
        p = r.astype(ml_dtypes.bfloat16)
        parts.append(p)
        r = r - p.astype(np.float32)
    return parts


def kernel(X, pos_emb, conv_w, bn_gamma, bn_beta, bn_mean, bn_var,
           _spmd_kwargs=None):
    X = np.ascontiguousarray(np.asarray(X, np.float32))
    zimg = _compute_z(pos_emb, conv_w, bn_gamma, bn_beta, bn_mean, bn_var)
    z_np = _quarter_major(zimg.reshape(NI, FREE))
    zparts = [np.ascontiguousarray(p) for p in _split_bf16(z_np)]

    nc = _build_kernel()
    in_maps = []
    for c in range(NCORES):
        shard = X[c * NB:(c + 1) * NB].reshape(NBLK, FREE)
        m = {"x": np.ascontiguousarray(shard), "s": _S_NP}
        for i, zp_ in enumerate(zparts):
            m[f"z{i}"] = zp_
        in_maps.append(m)

    res = run_bass_kernel_spmd(nc, in_maps, list(range(NCORES)),
                               **(_spmd_kwargs or {}))

    out = np.empty((B, N0, D0), np.float32)
    for c in range(NCORES):
        out[c * NB:(c + 1) * NB] = res.results[c]["out"].reshape(NB, N0, D0)
    if _spmd_kwargs:
        kernel.last_results = res
    return out


# revision 18
# speedup vs baseline: 1.0063x; 1.0063x over previous
"""Trainium2 kernel for nn_DeepPatchEncoder.

The reference pipeline (patchify16 + pos_emb -> unpatchify -> patchify8 +
pos_new -> unpatchify -> patchify16) collapses algebraically: patchify /
unpatchify are inverse permutations, so

    out = patchify16(X + Z),   Z = unpatchify16(pos_emb) + unpatchify8(pos_new)

where Z is a single [224,224,3] image computed from the tiny parameters
(pos_emb conv + batchnorm).  Z is computed on host in numpy (O(100KB) of
work); the per-sample memory-bound add + patch permutation runs on 8
NeuronCores, data-parallel over the batch (16 samples per core).

Per core the work is 224 independent blocks (sample b x coarse row i).
Block input = 16 consecutive image rows (10752 floats, contiguous in
DRAM); block output = 14 consecutive encoder rows (10752 floats,
contiguous in DRAM).  Within a block the map is a pure (p0:16 <-> j:14)
axis swap of 48-float chunks, done on the VectorEngine as tensor_tensor
adds with strided access patterns (which also add Z).

Engine / queue layout (all HWDGE DMAs share one SDMA queue row, so
side traffic must ride SWDGE to overlap the main read stream):
  - x loads: 4 contiguous 2.4MB DMAs (2 tiles x 2 p0-halves) on the SP
    HWDGE ring, issued first -- the only traffic on that queue row.
  - z replication (zrep[p] = z[p % 14] across 112 partitions): one-hot
    selection matmul on the otherwise-idle TensorEngine.  fp32 matmuls
    are ~5x slower than bf16, so the host splits z into two bf16
    components (z ~ z0 + z1, ~6e-7 relative on the output) and the PE
    accumulates the two exact bf16 products in PSUM; ScalarEngine
    copies PSUM->SBUF.  The host pre-permutes z into (j-half, p0-half)
    quarter-major order so each quarter is a contiguous column range:
    z component loads are contiguous, and each TT depends on exactly
    one quarter (so the DVE starts ~10us in, not after the full
    replication).  z loads ride SWDGE (gpsimd).
  - TTs: 8 per core ([112, 2688] each), consume (tile, j-half, p0-half).
  - stores: 4 contiguous 2.4MB DMAs (j-halves) on SWDGE, overlapping
    the read stream at the SDMA-engine round-robin level.
"""
import sys

for _p in ("/opt/trn_rl_repo", "/root/.axon_site/_ro/trn_rl_repo",
           "/root/.axon_site/_ro/pypackages"):
    if _p not in sys.path:
        sys.path.append(_p)

import numpy as np
import ml_dtypes
import concourse.bass as bass
import concourse.bacc as bacc
import concourse.mybir as mybir
import concourse.tile as tile
from concourse.bass_utils import run_bass_kernel_spmd

F32 = mybir.dt.float32
BF16 = mybir.dt.bfloat16

B, IMG, C = 128, 224, 3
P0, P1 = 16, 8
N0 = (IMG // P0) ** 2   # 196
D0 = C * P0 * P0        # 768
BN_EPS = 1e-3

NCORES = 8
NB = B // NCORES        # 16 samples per core
NI = IMG // P0          # 14 coarse rows
NBLK = NB * NI          # 224 blocks per core
ROWF = IMG * C          # 672 floats per image row
FREE = P0 * ROWF        # 10752 floats per block
P = 112                 # partitions per tile
NT = NBLK // P          # 2 tiles
NH = 2                  # j-halves (store / TT granularity)
JH = NI // NH           # 7
HFREE = FREE // NH      # 5376
NP0H = 2                # p0-halves (load / TT granularity)
P0H = P0 // NP0H        # 8
PHF = FREE // NP0H      # 5376 floats per p0-half (contiguous in x)
NQ = NH * NP0H          # 4 z quarters
QF = FREE // NQ         # 2688 floats per quarter
NZC = 2                 # bf16 z components
MMN = 512               # matmul moving-dim tile


def _compute_z(pos_emb, conv_w, bn_gamma, bn_beta, bn_mean, bn_var):
    """The [224,224,3] constant image Z (all-numpy, host side)."""
    pos_emb = np.asarray(pos_emb, np.float32)
    # unpatchify16(pos_emb): [196,768] -> [224,224,3]
    q = pos_emb.reshape(14, 14, P0, P0, C).transpose(0, 2, 1, 3, 4)
    q = q.reshape(IMG, IMG, C)

    # pos pipeline: [3,16,16,196] -conv2x2s2-> [3,8,8,784] -> BN
    pos_img = pos_emb.reshape(N0, P0, P0, C).transpose(3, 1, 2, 0)
    v = pos_img.reshape(C, 8, 2, 8, 2, N0).astype(np.float64)
    pos_c = np.einsum("nidjec,deco->nijo", v, np.asarray(conv_w, np.float64))
    inv = np.asarray(bn_gamma, np.float64) / np.sqrt(
        np.asarray(bn_var, np.float64) + BN_EPS)
    pos_c = (pos_c - np.asarray(bn_mean, np.float64)) * inv + np.asarray(
        bn_beta, np.float64)
    pos_new = pos_c.transpose(3, 1, 2, 0).astype(np.float32)  # [784,8,8,3]

    # unpatchify8(pos_new): [784,8,8,3] -> [224,224,3]
    r = pos_new.reshape(28, 28, P1, P1, C).transpose(0, 2, 1, 3, 4)
    r = r.reshape(IMG, IMG, C)
    return q + r


def _quarter_major(z):
    """[14, (p0:16, j:14, k:48)] -> [14, (h, ph, p0l:8, jl:7, k:48)].

    Quarter (h, ph) becomes the contiguous column range
    [(h*2+ph)*QF, (h*2+ph+1)*QF), laid out (p0l, jl, k)."""
    v = z.reshape(NI, NP0H, P0H, NH, JH, 48)        # i, ph, p0l, h, jl, k
    return np.ascontiguousarray(
        v.transpose(0, 3, 1, 2, 4, 5).reshape(NI, FREE))


_NC_CACHE = None


def _build_kernel():
    global _NC_CACHE
    if _NC_CACHE is not None:
        return _NC_CACHE
    nc = bacc.Bacc()
    x = nc.declare_dram_parameter("x", [NBLK, FREE], F32, isOutput=False)
    # z components, pre-permuted to quarter-major layout on host
    zs = [nc.declare_dram_parameter(f"z{i}", [NI, FREE], BF16, isOutput=False)
          for i in range(NZC)]
    s = nc.declare_dram_parameter("s", [NI, P], BF16, isOutput=False)
    out = nc.declare_dram_parameter("out", [NBLK, FREE], F32, isOutput=True)

    with tile.TileContext(nc) as tc:
        with (
            tc.tile_pool(name="cpool", bufs=1) as cpool,
            tc.tile_pool(name="zck", bufs=6) as zck,
            tc.tile_pool(name="zp", bufs=1) as zp,
            tc.tile_pool(name="ps", bufs=4, space="PSUM") as ps,
            tc.tile_pool(name="xp", bufs=2) as xp,
            tc.tile_pool(name="op", bufs=2) as op,
        ):
            # HWDGE (SP ring, queue row 1) carries the reads: the small
            # s/z-component loads sit at the head, interleaved with the
            # four fat x sub-loads so early z quarters arrive early.
            # (SWDGE's first DMA pays a ~13us GPSIMD library load, so the
            # latency-critical z loads must not ride it.)
            # tiny SWDGE warm-up DMA: the first gpsimd DMA triggers a ~10us
            # GPSIMD library load; absorb it at t=0 so the first real
            # store isn't delayed by it
            warm = cpool.tile([1, 16], BF16)
            nc.gpsimd.dma_start(out=warm[:], in_=s[0:1, 0:16])

            s_tile = cpool.tile([NI, P], BF16)
            nc.sync.dma_start(out=s_tile[:], in_=s[:, :])
            xts = [xp.tile([P, FREE], F32, tag="xt", name=f"xt{t}")
                   for t in range(NT)]
            zq_tiles = []
            zc_per_q = [[] for _ in range(NQ)]

            def load_zq(qi):
                for i in range(NZC):
                    zc = zck.tile([NI, QF], BF16, tag="zc")
                    nc.sync.dma_start(
                        out=zc[:], in_=zs[i][:, qi * QF:(qi + 1) * QF])
                    zc_per_q[qi].append(zc)

            def load_x(t, ph):
                nc.sync.dma_start(
                    out=xts[t][:, ph * PHF:(ph + 1) * PHF],
                    in_=x[t * P:(t + 1) * P, ph * PHF:(ph + 1) * PHF])

            # ring order: early z quarters first; later ones slotted
            # between the fat x sub-loads (zck has 6 slots so the q3
            # loads can't block the x loads queued behind them)
            load_zq(0)
            load_zq(1)
            load_x(0, 0)
            load_zq(2)
            load_x(0, 1)
            load_zq(3)
            load_x(1, 0)
            load_x(1, 1)

            # z replication (zrep[p] = z[p % 14]) on the TensorEngine:
            # psum[112, n] = S.T @ z_chunk (S one-hot bf16, exact),
            # accumulating the two bf16 z components.  Quarter at a time,
            # in TT consumption order.
            for qi in range(NQ):
                zqt = zp.tile([P, QF], F32, tag=f"zq{qi}")
                zq_tiles.append(zqt)
                zcs = zc_per_q[qi]
                for c0 in range(0, QF, MMN):
                    n = min(MMN, QF - c0)
                    pz = ps.tile([P, MMN], F32, tag="pz")
                    for i in range(NZC):
                        nc.tensor.matmul(pz[:, :n], s_tile[:],
                                         zcs[i][:, c0:c0 + n],
                                         start=(i == 0), stop=(i == NZC - 1))
                    nc.scalar.copy(out=zqt[:, c0:c0 + n], in_=pz[:, :n])

            for t in range(NT):
                xt = xts[t]
                for h in range(NH):
                    ot = op.tile([P, HFREE], F32, tag="ot")
                    for ph in range(NP0H):
                        # input view: (j:7, p0:8, k:48) strided over the
                        # p0-half of xt
                        in0 = xt[:].rearrange(
                            "p (p0 j k) -> p j p0 k", p0=P0, j=NI, k=48)[
                            :, h * JH:(h + 1) * JH,
                            ph * P0H:(ph + 1) * P0H]
                        # zrep quarter laid out (p0l:8, jl:7, k:48)
                        in1 = zq_tiles[h * NP0H + ph][:].rearrange(
                            "p (p0 j k) -> p j p0 k", p0=P0H, j=JH, k=48)
                        # output view inside the j-half tile
                        o0 = ot[:].rearrange(
                            "p (j p0 k) -> p j p0 k", j=JH, p0=P0, k=48)[
                            :, :, ph * P0H:(ph + 1) * P0H]
                        nc.vector.tensor_tensor(o0, in0, in1,
                                                mybir.AluOpType.add)
                    nc.gpsimd.dma_start(
                        out=out[t * P:(t + 1) * P, h * HFREE:(h + 1) * HFREE],
                        in_=ot[:])
    nc.finalize()
    _NC_CACHE = nc
    return nc


_S_NP = np.zeros((NI, P), ml_dtypes.bfloat16)
for _pp in range(P):
    _S_NP[_pp % NI, _pp] = 1.0


def _split_bf16(z, k=NZC):
    """z (f32) -> k bf16 arrays summing to z up to ~2^-(9k) relative."""
    parts = []
    r = z.astype(np.float32)
    for _ in range(k):
        p = r.astype(ml_dtypes.bfloat16)
        parts.append(p)
        r = r - p.astype(np.float32)
    return parts


def kernel(X, pos_emb, conv_w, bn_gamma, bn_beta, bn_mean, bn_var,
           _spmd_kwargs=None):
    X = np.ascontiguousarray(np.asarray(X, np.float32))
    zimg = _compute_z(pos_emb, conv_w, bn_gamma, bn_beta, bn_mean, bn_var)
    z_np = _quarter_major(zimg.reshape(NI, FREE))
    zparts = [np.ascontiguousarray(p) for p in _split_bf16(z_np)]

    nc = _build_kernel()
    in_maps = []
    for c in range(NCORES):
        shard = X[c * NB:(c + 1) * NB].reshape(NBLK, FREE)
        m = {"x": np.ascontiguousarray(shard), "s": _S_NP}
        for i, zp_ in enumerate(zparts):
            m[f"z{i}"] = zp_
        in_maps.append(m)

    res = run_bass_kernel_spmd(nc, in_maps, list(range(NCORES)),
                               **(_spmd_kwargs or {}))

    out = np.empty((B, N0, D0), np.float32)
    for c in range(NCORES):
        out[c * NB:(c + 1) * NB] = res.results[c]["out"].reshape(NB, N0, D0)
    if _spmd_kwargs:
        kernel.last_results = res
    return out
